# Initial kernel scaffold
#
"""Trainium2 Bass kernel for CausalSelfAttention (GQA + QK-RMSNorm + RoPE).

Problem shapes (hardcoded): B=2, S=2048, D=2048, H=16, KVH=4, HD=128.

Sharding: 8 cores = 2 batches x 4 kv-head groups. Core c handles batch
b = c // 4 and kv-group g = c % 4 (q-heads 4g..4g+3, kv head g).  Each core
computes its 4 heads end-to-end plus a partial output projection over its
512 columns of Wproj's input dim; the host sums the 4 partials per batch.

On-chip dataflow per core:
  Phase 1: stream xT, GEMM Q/K/V in [s, e] layout (fp32r), QK-RMSNorm +
           RoPE + gain on DVE/ACT, PE-transpose Q/K to [hd, s] layout.
  Phase 2: causal attention per (q-block 512, head): scores computed
           TRANSPOSED [k, q] so the softmax denominator comes from a
           ones-matmul (which also broadcasts it across partitions); exp on
           ACT; diagonal tiles masked with a triangular mask; PV matmul
           accumulates attnout^T [hd, q] over k-tiles in PSUM.
  Phase 3: partial out^T [e, s] = Wproj_slice^T-chunks @ y^T, DMA to HBM.
"""

import numpy as np

B, S, D = 2, 2048, 2048
H, KVH = 16, 4
HD = D // H            # 128
NH = H // KVH          # 4 heads per core
P = 128
ST = S // P            # 16 s-tiles
DT = D // P            # 16 d-tiles
FT = NH * HD // P      # 4 f-tiles (proj contraction per core)
QB = 512               # q-block width in phase 2
NQB = S // QB          # 4
SBW = 256              # phase-1 x DMA block width (s columns)
ROPE_BASE = 10000.0
EPS = 1e-6

_CACHE = {}


def _build_nc():
    from contextlib import ExitStack

    import concourse.mybir as mybir
    import concourse.tile as tile
    from concourse import bacc

    f32 = mybir.dt.float32
    f32r = mybir.dt.float32r
    AF = mybir.ActivationFunctionType
    MUL = mybir.AluOpType.mult
    ADD = mybir.AluOpType.add

    nc = bacc.Bacc("TRN2", target_bir_lowering=False, debug=False, num_devices=8)

    xT = nc.dram_tensor("xT", [D, S], f32r, kind="ExternalInput").ap()
    wqT = nc.dram_tensor("wqT", [D, NH * HD], f32r, kind="ExternalInput").ap()
    wkvT = nc.dram_tensor("wkvT", [D, 2 * HD], f32r, kind="ExternalInput").ap()
    wpT = nc.dram_tensor("wpT", [NH * HD, D], f32r, kind="ExternalInput").ap()
    cos2 = nc.dram_tensor("cos2", [S, HD], f32, kind="ExternalInput").ap()
    sin2 = nc.dram_tensor("sin2", [S, HD], f32, kind="ExternalInput").ap()
    qg4 = nc.dram_tensor("qg4", [P, NH], f32, kind="ExternalInput").ap()
    tri = nc.dram_tensor("tri", [P, P], f32r, kind="ExternalInput").ap()
    onesd = nc.dram_tensor("onesd", [P, P], f32r, kind="ExternalInput").ap()
    ident = nc.dram_tensor("ident", [P, P], f32, kind="ExternalInput").ap()
    outT = nc.dram_tensor("outT", [D, S], f32, kind="ExternalOutput").ap()

    with tile.TileContext(nc) as tc:
        with ExitStack() as octx:
            const = octx.enter_context(tc.tile_pool(name="const", bufs=1))
            big = octx.enter_context(tc.tile_pool(name="big", bufs=1))

            # ---- persistent stores ----
            QT = big.tile([P, NH, S], f32r)   # q^T per head: [hd, h, s]
            KT = big.tile([P, S], f32r)       # k^T: [hd, s]
            VS = big.tile([P, ST, HD], f32r)  # v: [s-part, s-tile, hd]
            YT = big.tile([P, NH, S], f32r)   # attn out^T per head: [hd, h, s]

            # =========================== Phase 1 ===========================
            with ExitStack() as ctx1:
                wpool = ctx1.enter_context(tc.tile_pool(name="wpool", bufs=1))
                xpool = ctx1.enter_context(tc.tile_pool(name="xpool", bufs=2))
                stq = ctx1.enter_context(tc.tile_pool(name="stq", bufs=2))
                stk = ctx1.enter_context(tc.tile_pool(name="stk", bufs=2))
                sml = ctx1.enter_context(tc.tile_pool(name="sml", bufs=2))
                ps_q = ctx1.enter_context(
                    tc.tile_pool(name="ps_q", bufs=2, space="PSUM"))
                ps_kv = ctx1.enter_context(
                    tc.tile_pool(name="ps_kv", bufs=2, space="PSUM"))
                ps_tr = ctx1.enter_context(
                    tc.tile_pool(name="ps_tr", bufs=3, space="PSUM"))

                xTr = xT.rearrange("(dt p) s -> p dt s", p=P)
                wqTr = wqT.rearrange("(dt p) e -> p dt e", p=P)
                wkvTr = wkvT.rearrange("(dt p) e -> p dt e", p=P)

                # Chunked DMAs so the first matmuls' deps land early.
                def load_xblk(sb):
                    t = xpool.tile([P, DT, SBW], f32r, tag="xblk", name="xblk")
                    for c in range(0, DT, 2):
                        nc.sync.dma_start(
                            t[:, c:c + 2, :],
                            xTr[:, c:c + 2, sb * SBW:(sb + 1) * SBW])
                    return t

                xblk_next = load_xblk(0)

                WQ = wpool.tile([P, DT, NH * HD], f32r)
                WKV = wpool.tile([P, DT, 2 * HD], f32r)
                for c in range(0, DT, 4):
                    nc.sync.dma_start(WKV[:, c:c + 4, :], wkvTr[:, c:c + 4, :])
                for c in range(0, DT, 2):
                    nc.sync.dma_start(WQ[:, c:c + 2, :], wqTr[:, c:c + 2, :])

                # ---- constants (needed a few microseconds in) ----
                cos_t = const.tile([P, ST, HD], f32)
                sin_t = const.tile([P, ST, HD], f32)
                cos2r = cos2.rearrange("(st p) c -> p st c", p=P)
                sin2r = sin2.rearrange("(st p) c -> p st c", p=P)
                qg_t = const.tile([P, NH], f32)
                nc.sync.dma_start(qg_t[:], qg4)
                tri_t = const.tile([P, P], f32r)
                nc.sync.dma_start(tri_t[:], tri)
                ones_t = const.tile([P, P], f32r)
                nc.sync.dma_start(ones_t[:], onesd)
                id_t = const.tile([P, P], f32)
                nc.sync.dma_start(id_t[:], ident)
                idr_t = const.tile([P, P], f32r)
                nc.sync.dma_start(idr_t[:], ident.bitcast(f32r))
                eps_t = const.tile([P, 1], f32)
                nc.vector.memset(eps_t[:], EPS)

                nsb = SBW // P
                for sb in range(S // SBW):
                    xblk = xblk_next
                    if sb + 1 < S // SBW:
                        xblk_next = load_xblk(sb + 1)
                    nc.sync.dma_start(cos_t[:, sb * nsb:(sb + 1) * nsb, :],
                                      cos2r[:, sb * nsb:(sb + 1) * nsb, :])
                    nc.sync.dma_start(sin_t[:, sb * nsb:(sb + 1) * nsb, :],
                                      sin2r[:, sb * nsb:(sb + 1) * nsb, :])
                    for jj in range(SBW // P):
                        st = sb * (SBW // P) + jj
                        xs = xblk[:, :, jj * P:(jj + 1) * P]

                        psq = ps_q.tile([P, NH * HD], f32)
                        for dt in range(DT):
                            nc.tensor.matmul(
                                psq[:], xs[:, dt],
                                WQ[:, dt],
                                start=(dt == 0), stop=(dt == DT - 1))
                        pskv = ps_kv.tile([P, 2 * HD], f32)
                        for dt in range(DT):
                            nc.tensor.matmul(
                                pskv[:], xs[:, dt],
                                WKV[:, dt],
                                start=(dt == 0), stop=(dt == DT - 1))

                        # V straight to its store
                        nc.scalar.copy(VS[:, st], pskv[:, HD:2 * HD])

                        # -- Q rmsnorm + rope + gain --
                        psq3 = psq[:].rearrange("p (h c) -> p h c", h=NH)
                        ssq = sml.tile([P, NH], f32, tag="ssq")
                        sqscr = stq.tile([P, NH, HD], f32, tag="qa")
                        for h in range(NH):
                            nc.scalar.activation(sqscr[:, h], psq3[:, h], AF.Square,
                                                 accum_out=ssq[:, h:h + 1])
                        msq = sml.tile([P, NH], f32, tag="msq")
                        nc.scalar.activation(msq[:], ssq[:], AF.Sqrt,
                                             bias=eps_t[:], scale=1.0 / HD)
                        rsq = sml.tile([P, NH], f32, tag="rsq")
                        nc.vector.reciprocal(rsq[:], msq[:])
                        rsg = sml.tile([P, NH], f32, tag="rsg")
                        nc.vector.tensor_tensor(rsg[:], rsq[:], qg_t[:], MUL)

                        qn = stq.tile([P, NH, HD], f32, tag="qn")
                        nc.vector.tensor_tensor(
                            qn[:], psq3, rsg[:, :, None].to_broadcast([P, NH, HD]),
                            MUL)
                        qa = stq.tile([P, NH, HD], f32, tag="qa")
                        nc.vector.tensor_tensor(
                            qa[:], qn[:],
                            cos_t[:, st:st + 1, :].to_broadcast([P, NH, HD]), MUL)
                        qb = stq.tile([P, NH, HD], f32, tag="qb")
                        nc.vector.tensor_tensor(
                            qb[:, :, 0:HD // 2], qn[:, :, HD // 2:HD],
                            sin_t[:, st:st + 1, 0:HD // 2].to_broadcast(
                                [P, NH, HD // 2]), MUL)
                        nc.vector.tensor_tensor(
                            qb[:, :, HD // 2:HD], qn[:, :, 0:HD // 2],
                            sin_t[:, st:st + 1, HD // 2:HD].to_broadcast(
                                [P, NH, HD // 2]), MUL)
                        qrot = stq.tile([P, NH, HD], f32, tag="qn")
                        nc.vector.tensor_tensor(qrot[:], qa[:], qb[:], ADD)

                        # -- K rmsnorm + rope --
                        ssk = sml.tile([P, 1], f32, tag="ssk")
                        skscr = stk.tile([P, HD], f32, tag="ka")
                        nc.scalar.activation(skscr[:], pskv[:, 0:HD], AF.Square,
                                             accum_out=ssk[:])
                        msk = sml.tile([P, 1], f32, tag="msk")
                        nc.scalar.activation(msk[:], ssk[:], AF.Sqrt,
                                             bias=eps_t[:], scale=1.0 / HD)
                        rsk = sml.tile([P, 1], f32, tag="rsk")
                        nc.vector.reciprocal(rsk[:], msk[:])

                        kn = stk.tile([P, HD], f32, tag="kn")
                        nc.vector.tensor_tensor(
                            kn[:], pskv[:, 0:HD], rsk[:].to_broadcast([P, HD]), MUL)
                        ka = stk.tile([P, HD], f32, tag="ka")
                        nc.gpsimd.tensor_tensor(ka[:], kn[:], cos_t[:, st], MUL)
                        kb = stk.tile([P, HD], f32, tag="kb")
                        nc.gpsimd.tensor_tensor(
                            kb[:, 0:HD // 2], kn[:, HD // 2:HD],
                            sin_t[:, st, 0:HD // 2], MUL)
                        nc.gpsimd.tensor_tensor(
                            kb[:, HD // 2:HD], kn[:, 0:HD // 2],
                            sin_t[:, st, HD // 2:HD], MUL)
                        krot = stk.tile([P, HD], f32, tag="kn")
                        nc.gpsimd.tensor_tensor(krot[:], ka[:], kb[:], ADD)

                        # -- transposes into QT / KT --
                        for h in range(NH):
                            ptr = ps_tr.tile([P, P], f32, tag="tr")
                            nc.tensor.transpose(ptr[:], qrot[:, h], id_t[:])
                            if h % 2 == 0:
                                nc.scalar.copy(QT[:, h, st * P:(st + 1) * P], ptr[:])
                            else:
                                nc.vector.tensor_copy(
                                    QT[:, h, st * P:(st + 1) * P], ptr[:])
                        ptrk = ps_tr.tile([P, P], f32, tag="tr")
                        nc.tensor.transpose(ptrk[:], krot[:], id_t[:])
                        nc.scalar.copy(KT[:, st * P:(st + 1) * P], ptrk[:])

            # ======================= Phases 2 and 3 ========================
            with ExitStack() as ctx2:
                wp2 = ctx2.enter_context(tc.tile_pool(name="wp2", bufs=1))
                ostage = ctx2.enter_context(tc.tile_pool(name="ostage", bufs=6))

                WP = wp2.tile([P, FT, D], f32r)
                nc.sync.dma_start(WP[:], wpT.rearrange("(ft p) e -> p ft e", p=P))

                # ------------------ Phases 2 + 3 interleaved ---------------
                with ExitStack() as ctx2b:
                    expool = ctx2b.enter_context(
                        tc.tile_pool(name="expool", bufs=12))
                    recpool = ctx2b.enter_context(
                        tc.tile_pool(name="recpool", bufs=3))
                    ps_s = ctx2b.enter_context(
                        tc.tile_pool(name="ps_s", bufs=4, space="PSUM"))
                    ps_o = ctx2b.enter_context(
                        tc.tile_pool(name="ps_o", bufs=1, space="PSUM"))
                    ps_d = ctx2b.enter_context(
                        tc.tile_pool(name="ps_d", bufs=1, space="PSUM"))
                    ps_p3 = ctx2b.enter_context(
                        tc.tile_pool(name="ps_p3", bufs=2, space="PSUM"))

                    for qb in range(NQB):
                        for h in range(NH):
                            oT = ps_o.tile([P, QB], f32)
                            den = ps_d.tile([P, QB], f32)
                            nk = NH * qb + NH
                            grp_start, grp_sum, grp_n = None, None, 0
                            for kt in range(nk):
                                j = kt - NH * qb  # >= 0 on diagonal tiles
                                q0 = P * j if j >= 0 else 0
                                ps = ps_s.tile([P, QB], f32)
                                nc.tensor.matmul(
                                    ps[:, q0:QB],
                                    KT[:, kt * P:(kt + 1) * P],
                                    QT[:, h, qb * QB + q0:(qb + 1) * QB],
                                    start=True, stop=True)
                                ex = expool.tile([P, QB], f32r, tag="ex")
                                if j >= 1:
                                    nc.gpsimd.memset(
                                        ex[:, 0:q0].bitcast(f32), 0.0)
                                nc.scalar.activation(
                                    ex[:, q0:QB], ps[:, q0:QB], AF.Exp)
                                if j >= 0:
                                    nc.vector.tensor_tensor(
                                        ex[:, q0:q0 + P], ex[:, q0:q0 + P],
                                        tri_t[:], MUL)
                                nc.tensor.matmul(
                                    oT[:, q0:QB], VS[:, kt], ex[:, q0:QB],
                                    start=(kt == 0), stop=(kt == nk - 1))
                                # denominator: running-sum groups of full tiles
                                # on DVE so the ones-matmul runs once per 4
                                # k-tiles on PE (the bottleneck engine)
                                if j < 0:
                                    if grp_sum is None:
                                        grp_start, grp_sum, grp_n = kt, ex, 1
                                    else:
                                        ns = expool.tile(
                                            [P, QB], f32r, tag="exs")
                                        nc.vector.tensor_tensor(
                                            ns[:], grp_sum[:], ex[:], ADD)
                                        grp_sum = ns
                                        grp_n += 1
                                    if grp_n == 4 or kt + 1 >= NH * qb:
                                        nc.tensor.matmul(
                                            den[:], ones_t[:], grp_sum[:],
                                            start=(grp_start == 0),
                                            stop=False)
                                        grp_sum, grp_n = None, 0
                                else:
                                    nc.tensor.matmul(
                                        den[:, q0:QB], ones_t[:], ex[:, q0:QB],
                                        start=(kt == 0), stop=(kt == nk - 1))
                            rec = recpool.tile([P, QB], f32, tag="rec")
                            nc.vector.reciprocal_approx_fast(rec[:], den[:])
                            nc.vector.tensor_tensor(
                                YT[:, h, qb * QB:(qb + 1) * QB], oT[:], rec[:],
                                MUL)

                        # phase-3 for this s-block (= qb) fills PE gaps
                        sb3 = qb
                        for et in range(DT):
                            po = ps_p3.tile([P, QB], f32)
                            for ft in range(FT):
                                nc.tensor.matmul(
                                    po[:],
                                    WP[:, ft, et * P:(et + 1) * P],
                                    YT[:, ft, sb3 * QB:(sb3 + 1) * QB],
                                    start=(ft == 0), stop=(ft == FT - 1))
                            ob = ostage.tile([P, QB], f32)
                            if et % 2 == 0:
                                nc.scalar.copy(ob[:], po[:])
                            else:
                                nc.vector.tensor_copy(ob[:], po[:])
                            nc.sync.dma_start(
                                outT[et * P:(et + 1) * P,
                                     sb3 * QB:(sb3 + 1) * QB], ob[:])

    nc.compile()
    return nc


def _host_inputs(x, Wq, Wk, Wv, Wproj, q_gain):
    """Build the 8 per-core input maps."""
    f32 = np.float32
    inv_freq = 1.0 / (ROPE_BASE ** (np.arange(0, HD, 2, dtype=f32) / HD))
    freqs = np.outer(np.arange(S, dtype=f32), inv_freq).astype(f32)
    c = np.cos(freqs).astype(f32)
    s = np.sin(freqs).astype(f32)
    cos2 = np.concatenate([c, c], axis=1)
    sin2 = np.concatenate([s, -s], axis=1)
    tri = np.triu(np.ones((P, P), dtype=f32))          # tri[k, q] = k <= q
    onesd = np.ones((P, P), dtype=f32)
    ident = np.eye(P, dtype=f32)

    in_maps = []
    for core in range(8):
        b, g = divmod(core, KVH)
        hs = g * NH * HD            # first q row for this group
        qg = (q_gain[g * NH:(g + 1) * NH].astype(f32) * (HD ** -0.5))
        in_maps.append({
            "xT": np.ascontiguousarray(x[b].T, dtype=f32),
            "wqT": np.ascontiguousarray(Wq[hs:hs + NH * HD].T, dtype=f32),
            "wkvT": np.ascontiguousarray(
                np.concatenate([Wk[g * HD:(g + 1) * HD], Wv[g * HD:(g + 1) * HD]],
                               axis=0).T, dtype=f32),
            "wpT": np.ascontiguousarray(Wproj.T[hs:hs + NH * HD], dtype=f32),
            "cos2": cos2, "sin2": sin2,
            "qg4": np.ascontiguousarray(np.broadcast_to(qg, (P, NH)), dtype=f32),
            "tri": tri, "onesd": onesd, "ident": ident,
        })
    return in_maps


def kernel(x, Wq, Wk, Wv, Wproj, q_gain):
    from concourse.bass_utils import run_bass_kernel_spmd

    x = np.asarray(x, dtype=np.float32)
    Wq = np.asarray(Wq, dtype=np.float32)
    Wk = np.asarray(Wk, dtype=np.float32)
    Wv = np.asarray(Wv, dtype=np.float32)
    Wproj = np.asarray(Wproj, dtype=np.float32)
    q_gain = np.asarray(q_gain, dtype=np.float32)

    if "nc" not in _CACHE:
        _CACHE["nc"] = _build_nc()
    nc = _CACHE["nc"]

    in_maps = _host_inputs(x, Wq, Wk, Wv, Wproj, q_gain)
    res = run_bass_kernel_spmd(nc, in_maps, core_ids=list(range(8)))

    out = np.zeros((B, S, D), dtype=np.float32)
    for core in range(8):
        b = core // KVH
        out[b] += res.results[core]["outT"].T
    return out



# revision 40
# speedup vs baseline: 1.0078x; 1.0078x over previous
"""Trainium2 Bass kernel for CausalSelfAttention (GQA + QK-RMSNorm + RoPE).

Problem shapes (hardcoded): B=2, S=2048, D=2048, H=16, KVH=4, HD=128.

Sharding: 8 cores = 2 batches x 4 kv-head groups. Core c handles batch
b = c // 4 and kv-group g = c % 4 (q-heads 4g..4g+3, kv head g).  Each core
computes its 4 heads end-to-end plus a partial output projection over its
512 columns of Wproj's input dim; the host sums the 4 partials per batch.

v2: all-bf16 matmul datapath (inputs converted on host), paired exp tiles,
grouped softmax denominators incl. diagonals, engine rebalance
(ACT per-partition-scale for rmsnorm scaling, DVE tensor_tensor_reduce for
squares, Pool for k-rope/copies).

On-chip dataflow per core:
  Phase 1: stream xT (bf16), GEMM Q/K/V in [s, e] layout, QK-RMSNorm +
           RoPE + gain, PE-transpose Q/K to [hd, s] bf16 layout.
  Phase 2: causal attention per (q-block 512, head): scores TRANSPOSED
           [k, q] in paired PSUM tiles [128, 2, 512]; exp once per pair on
           ACT -> bf16; diagonal tiles masked; PV accumulates oT [hd, q];
           denominator via ones-matmul over DVE-summed groups of 4 tiles.
  Phase 3: partial out^T [e, s] = Wproj_slice^T-chunks @ y^T, bf16 to HBM.
"""

import numpy as np

B, S, D = 2, 2048, 2048
H, KVH = 16, 4
HD = D // H            # 128
NH = H // KVH          # 4 heads per core
P = 128
ST = S // P            # 16 s-tiles
DT = D // P            # 16 d-tiles
FT = NH * HD // P      # 4 f-tiles (proj contraction per core)
QB = 512               # q-block width in phase 2
NQB = S // QB          # 4
SBW = 512              # phase-1 x DMA block width (s columns)
ROPE_BASE = 10000.0
EPS = 1e-6
FSC = 32.0             # fp8 per-tensor scale for x and W (psum = 1024x)

_CACHE = {}


def _build_nc():
    from contextlib import ExitStack

    import concourse.mybir as mybir
    import concourse.tile as tile
    from concourse import bacc

    f32 = mybir.dt.float32
    bf16 = mybir.dt.bfloat16
    f8 = mybir.dt.float8e4
    AF = mybir.ActivationFunctionType
    MUL = mybir.AluOpType.mult
    ADD = mybir.AluOpType.add
    DR = mybir.MatmulPerfMode.DoubleRow

    nc = bacc.Bacc("TRN2", target_bir_lowering=False, debug=False, num_devices=8)

    xTh = nc.dram_tensor("xTh", [D, S], f8, kind="ExternalInput").ap()
    xTl = nc.dram_tensor("xTl", [D, S], f8, kind="ExternalInput").ap()
    wqTh = nc.dram_tensor("wqTh", [D, NH * HD], f8, kind="ExternalInput").ap()
    wqTl = nc.dram_tensor("wqTl", [D, NH * HD], f8, kind="ExternalInput").ap()
    wkvTh = nc.dram_tensor("wkvTh", [D, 2 * HD], f8, kind="ExternalInput").ap()
    wkvTl = nc.dram_tensor("wkvTl", [D, 2 * HD], f8, kind="ExternalInput").ap()
    wpT = nc.dram_tensor("wpT", [NH * HD, D], bf16, kind="ExternalInput").ap()
    cos2 = nc.dram_tensor("cos2", [S, HD], bf16, kind="ExternalInput").ap()
    sin2 = nc.dram_tensor("sin2", [S, HD], bf16, kind="ExternalInput").ap()
    qg5 = nc.dram_tensor("qg5", [P, NH + 1], f32, kind="ExternalInput").ap()
    tri = nc.dram_tensor("tri", [P, P], bf16, kind="ExternalInput").ap()
    onesd = nc.dram_tensor("onesd", [P, P], bf16, kind="ExternalInput").ap()
    ident = nc.dram_tensor("ident", [P, P], bf16, kind="ExternalInput").ap()
    outT = nc.dram_tensor("outT", [D, S], bf16, kind="ExternalOutput").ap()

    with tile.TileContext(nc) as tc:
        with ExitStack() as octx:
            const = octx.enter_context(tc.tile_pool(name="const", bufs=1))
            big = octx.enter_context(tc.tile_pool(name="big", bufs=1))

            # ---- persistent stores (split per s-block so phase-2/3 deps
            # are range-accurate instead of whole-tensor) ----
            QTB = [big.tile([P, NH, QB], bf16, name=f"QTB{b}")
                   for b in range(NQB)]       # q^T per head: [hd, h, s-blk]
            KTB = [big.tile([P, QB], bf16, name=f"KTB{b}")
                   for b in range(NQB)]       # k^T: [hd, s-blk]
            VS = big.tile([P, ST, HD], bf16)  # v: [s-part, s-tile, hd]
            YTB = [big.tile([P, NH, QB], bf16, name=f"YTB{b}")
                   for b in range(NQB)]       # attn out^T: [hd, h, s-blk]

            # =========================== Phase 1 ===========================
            with ExitStack() as ctx1:
                wpool = ctx1.enter_context(tc.tile_pool(name="wpool", bufs=1))
                xpool = ctx1.enter_context(tc.tile_pool(name="xpool", bufs=3))
                stq = ctx1.enter_context(tc.tile_pool(name="stq", bufs=2))
                stk = ctx1.enter_context(tc.tile_pool(name="stk", bufs=2))
                sml = ctx1.enter_context(tc.tile_pool(name="sml", bufs=2))
                ps_q = ctx1.enter_context(
                    tc.tile_pool(name="ps_q", bufs=3, space="PSUM"))
                ps_kv = ctx1.enter_context(
                    tc.tile_pool(name="ps_kv", bufs=2, space="PSUM"))
                ps_tr = ctx1.enter_context(
                    tc.tile_pool(name="ps_tr", bufs=3, space="PSUM"))

                xThr = xTh.rearrange("(dt p) s -> p dt s", p=P)
                xTlr = xTl.rearrange("(dt p) s -> p dt s", p=P)
                wqThr = wqTh.rearrange("(dt p) e -> p dt e", p=P)
                wqTlr = wqTl.rearrange("(dt p) e -> p dt e", p=P)
                wkvThr = wkvTh.rearrange("(dt p) e -> p dt e", p=P)
                wkvTlr = wkvTl.rearrange("(dt p) e -> p dt e", p=P)

                # Coarse DMAs: each dma_start costs ~565ns of SP sequencer
                # time, so issue few large transfers.
                def load_xblk(sb):
                    th = xpool.tile([P, DT, SBW], f8, tag="xh", name="xh")
                    tl = xpool.tile([P, DT, SBW], f8, tag="xl", name="xl")
                    nc.sync.dma_start(
                        th[:], xThr[:, :, sb * SBW:(sb + 1) * SBW])
                    nc.sync.dma_start(
                        tl[:], xTlr[:, :, sb * SBW:(sb + 1) * SBW])
                    return th, tl

                # Startup: interleave x-block-0 halves with weights so the
                # first matmuls' deps land early.
                WQh = wpool.tile([P, DT, NH * HD], f8)
                WQl = wpool.tile([P, DT, NH * HD], f8)
                WKVh = wpool.tile([P, DT, 2 * HD], f8)
                WKVl = wpool.tile([P, DT, 2 * HD], f8)
                xh0 = xpool.tile([P, DT, SBW], f8, tag="xh", name="xh")
                xl0 = xpool.tile([P, DT, SBW], f8, tag="xl", name="xl")
                nc.sync.dma_start(xh0[:, 0:4, :], xThr[:, 0:4, 0:SBW])
                nc.sync.dma_start(WQh[:, 0:4, :], wqThr[:, 0:4, :])
                nc.sync.dma_start(xh0[:, 4:8, :], xThr[:, 4:8, 0:SBW])
                nc.sync.dma_start(WQh[:, 4:8, :], wqThr[:, 4:8, :])
                nc.sync.dma_start(xh0[:, 8:16, :], xThr[:, 8:16, 0:SBW])
                nc.sync.dma_start(WQh[:, 8:16, :], wqThr[:, 8:16, :])
                nc.sync.dma_start(WKVh[:], wkvThr[:])
                nc.sync.dma_start(xl0[:, 0:8, :], xTlr[:, 0:8, 0:SBW])
                nc.sync.dma_start(xl0[:, 8:16, :], xTlr[:, 8:16, 0:SBW])
                nc.sync.dma_start(WQl[:], wqTlr[:])
                nc.sync.dma_start(WKVl[:], wkvTlr[:])

                # ---- constants, ordered by first use ----
                cos_t = const.tile([P, ST, HD], bf16)
                sin_t = const.tile([P, ST, HD], bf16)
                cos2r = cos2.rearrange("(st p) c -> p st c", p=P)
                sin2r = sin2.rearrange("(st p) c -> p st c", p=P)
                nc.sync.dma_start(cos_t[:, 0:2, :], cos2r[:, 0:2, :])
                nc.sync.dma_start(sin_t[:, 0:2, :], sin2r[:, 0:2, :])
                qg_t = const.tile([P, NH + 1], f32)
                nc.sync.dma_start(qg_t[:], qg5)
                id_t = const.tile([P, P], bf16)
                nc.sync.dma_start(id_t[:], ident)
                # prefetch x-block 1 ahead of the phase-2-only constants
                xblk_nn = load_xblk(1)
                xblk_next = (xh0, xl0)
                nc.sync.dma_start(cos_t[:, 2:16, :], cos2r[:, 2:16, :])
                nc.sync.dma_start(sin_t[:, 2:16, :], sin2r[:, 2:16, :])
                tri_t = const.tile([P, P], bf16)
                nc.sync.dma_start(tri_t[:], tri)
                ones_t = const.tile([P, P], bf16)
                nc.sync.dma_start(ones_t[:], onesd)
                eps_t = const.tile([P, 1], f32)
                nc.vector.memset(eps_t[:], EPS)

                # Transposes run one s-tile late so PE never waits on the
                # rmsnorm/rope element-wise chain.
                def do_transposes(qrot, krot, st):
                    blk, off = st // 4, (st % 4) * P
                    for h in range(NH):
                        ptr = ps_tr.tile([P, P], bf16, tag="tr", name="ptr")
                        nc.tensor.transpose(ptr[:], qrot[:, h], id_t[:])
                        if h < 3:
                            nc.vector.tensor_copy(
                                QTB[blk][:, h, off:off + P], ptr[:])
                        else:
                            nc.gpsimd.tensor_copy(
                                QTB[blk][:, h, off:off + P], ptr[:])
                    ptrk = ps_tr.tile([P, P], bf16, tag="tr", name="ptrk")
                    nc.tensor.transpose(ptrk[:], krot[:], id_t[:])
                    nc.scalar.copy(KTB[blk][:, off:off + P], ptrk[:])

                def qmm(ps, xt, wt, sl, start, stop):
                    for dt in range(0, DT, 2):
                        nc.tensor.matmul(
                            ps[:], xt[:, dt:dt + 2, sl], wt[:, dt:dt + 2, :],
                            start=(start and dt == 0),
                            stop=(stop and dt == DT - 2), perf_mode=DR)

                xq = [xblk_next, xblk_nn]
                prev_rot = None

                def do_tile(st, xh, xl, jj, psq=None, pskv=None):
                    nonlocal prev_rot
                    sl = slice(jj * P, (jj + 1) * P)
                    # compensated fp8: q*1024 = Xh@Wh + Xl@Wh + Xh@Wl
                    # (same scale on all three terms), DoubleRow pairs.
                    if psq is None:
                        psq = ps_q.tile([P, NH * HD], f32, name="psq")
                        qmm(psq, xh, WQh, sl, True, False)
                    qmm(psq, xl, WQh, sl, False, False)
                    qmm(psq, xh, WQl, sl, False, True)
                    if pskv is None:
                        pskv = ps_kv.tile([P, 2 * HD], f32, name="pskv")
                        qmm(pskv, xh, WKVh, sl, True, False)
                    qmm(pskv, xl, WKVh, sl, False, False)
                    qmm(pskv, xh, WKVl, sl, False, True)
                    if prev_rot is not None:
                        do_transposes(*prev_rot)
                    elementwise(st, psq, pskv)

                def elementwise(st, psq, pskv):
                    nonlocal prev_rot
                    if True:
                        # Last two s-tiles: copy PSUM off to SBUF right away
                        # so phase-2 pools inherit drained banks.
                        if st >= ST - 2:
                            psq_s = stq.tile([P, NH * HD], f32, tag="psqs")
                            nc.vector.tensor_copy(psq_s[:], psq[:])
                            pskv_s = stk.tile([P, 2 * HD], f32, tag="pskvs")
                            nc.scalar.copy(pskv_s[:], pskv[:])
                            psq, pskv = psq_s, pskv_s

                        # V straight to its store (Pool), undo fp8 scale
                        nc.gpsimd.tensor_scalar_mul(
                            VS[:, st], pskv[:, HD:2 * HD], 1.0 / 1024)

                        # -- sum of squares on ACT (hw allows only one PSUM
                        # operand per DVE/Pool instruction) --
                        psq3 = psq[:].rearrange("p (h c) -> p h c", h=NH)
                        ssq = sml.tile([P, NH + 1], f32, tag="ssq")
                        sqd = stq.tile([P, NH + 1, HD], bf16, tag="sqd")
                        for h in range(NH):
                            nc.scalar.activation(sqd[:, h], psq3[:, h],
                                                 AF.Square,
                                                 accum_out=ssq[:, h:h + 1])
                        nc.scalar.activation(sqd[:, NH], pskv[:, 0:HD],
                                             AF.Square,
                                             accum_out=ssq[:, NH:NH + 1])

                        # rms = sqrt(ss/HD + eps); rsg = qg / rms (col 4 = k)
                        msq = sml.tile([P, NH + 1], f32, tag="msq")
                        nc.scalar.activation(msq[:], ssq[:], AF.Sqrt,
                                             bias=eps_t[:],
                                             scale=1.0 / (HD * 1024.0 * 1024.0))
                        rsq = sml.tile([P, NH + 1], f32, tag="rsq")
                        nc.vector.reciprocal(rsq[:], msq[:])
                        rsg = sml.tile([P, NH + 1], f32, tag="rsg")
                        nc.vector.tensor_tensor(rsg[:], rsq[:], qg_t[:], MUL)

                        # -- Q normalize (DVE: one PSUM operand) + rope --
                        qn = stq.tile([P, NH, HD], bf16, tag="qn")
                        nc.vector.tensor_tensor(
                            qn[:], psq3,
                            rsg[:, 0:NH, None].to_broadcast([P, NH, HD]),
                            MUL)
                        qa = stq.tile([P, NH, HD], bf16, tag="qa")
                        nc.vector.tensor_tensor(
                            qa[:], qn[:],
                            cos_t[:, st:st + 1, :].to_broadcast([P, NH, HD]), MUL)
                        qb = stq.tile([P, NH, HD], bf16, tag="qb")
                        nc.vector.tensor_tensor(
                            qb[:, :, 0:HD // 2], qn[:, :, HD // 2:HD],
                            sin_t[:, st:st + 1, 0:HD // 2].to_broadcast(
                                [P, NH, HD // 2]), MUL)
                        nc.vector.tensor_tensor(
                            qb[:, :, HD // 2:HD], qn[:, :, 0:HD // 2],
                            sin_t[:, st:st + 1, HD // 2:HD].to_broadcast(
                                [P, NH, HD // 2]), MUL)
                        qrot = stq.tile([P, NH, HD], bf16, tag="qn2")
                        nc.vector.tensor_tensor(qrot[:], qa[:], qb[:], ADD)

                        # -- K normalize (DVE) + rope (Pool) --
                        kn = stk.tile([P, HD], bf16, tag="kn")
                        nc.vector.tensor_tensor(
                            kn[:], pskv[:, 0:HD],
                            rsg[:, NH:NH + 1].to_broadcast([P, HD]), MUL)
                        ka = stk.tile([P, HD], bf16, tag="ka")
                        nc.gpsimd.tensor_tensor(ka[:], kn[:], cos_t[:, st], MUL)
                        kb = stk.tile([P, HD], bf16, tag="kb")
                        nc.gpsimd.tensor_tensor(
                            kb[:, 0:HD // 2], kn[:, HD // 2:HD],
                            sin_t[:, st, 0:HD // 2], MUL)
                        nc.gpsimd.tensor_tensor(
                            kb[:, HD // 2:HD], kn[:, 0:HD // 2],
                            sin_t[:, st, HD // 2:HD], MUL)
                        krot = stk.tile([P, HD], bf16, tag="kn2")
                        nc.gpsimd.tensor_tensor(krot[:], ka[:], kb[:], ADD)

                        prev_rot = (qrot, krot, st)
                do_transposes(*prev_rot)

            # ======================= Phases 2 and 3 ========================
            with ExitStack() as ctx2:
                wp2 = ctx2.enter_context(tc.tile_pool(name="wp2", bufs=1))
                ostage = ctx2.enter_context(tc.tile_pool(name="ostage", bufs=6))

                WP = wp2.tile([P, FT, D], bf16)
                nc.sync.dma_start(WP[:], wpT.rearrange("(ft p) e -> p ft e", p=P))

                # ------------------ Phases 2 + 3 interleaved ---------------
                with ExitStack() as ctx2b:
                    expool = ctx2b.enter_context(
                        tc.tile_pool(name="expool", bufs=8))
                    sumpool = ctx2b.enter_context(
                        tc.tile_pool(name="sumpool", bufs=4))
                    recpool = ctx2b.enter_context(
                        tc.tile_pool(name="recpool", bufs=3))
                    ps_s = ctx2b.enter_context(
                        tc.tile_pool(name="ps_s", bufs=2, space="PSUM"))
                    ps_o = ctx2b.enter_context(
                        tc.tile_pool(name="ps_o", bufs=1, space="PSUM"))
                    ps_d = ctx2b.enter_context(
                        tc.tile_pool(name="ps_d", bufs=1, space="PSUM"))
                    ps_p3 = ctx2b.enter_context(
                        tc.tile_pool(name="ps_p3", bufs=2, space="PSUM"))

                    def p3_unit(sb3, et):
                        po = ps_p3.tile([P, QB], f32, name="po")
                        for ft in range(FT):
                            nc.tensor.matmul(
                                po[:],
                                WP[:, ft, et * P:(et + 1) * P],
                                YTB[sb3][:, ft, :],
                                start=(ft == 0), stop=(ft == FT - 1))
                        ob = ostage.tile([P, QB], bf16, name="ob")
                        if et % 3 == 0:
                            nc.scalar.copy(ob[:], po[:])
                        elif et % 3 == 1:
                            nc.vector.tensor_copy(ob[:], po[:])
                        else:
                            nc.gpsimd.tensor_copy(ob[:], po[:])
                        nc.sync.dma_start(
                            outT[et * P:(et + 1) * P,
                                 sb3 * QB:(sb3 + 1) * QB], ob[:])

                    for qb in range(NQB):
                        for h in range(NH):
                            oT = ps_o.tile([P, QB], f32)
                            den = ps_d.tile([P, QB], f32)
                            nk = NH * qb + NH
                            # groups of 4 k-tiles -> one den matmul each
                            ngrp = nk // 4
                            ex_tiles = []

                            for pr in range(nk // 2):
                                k0, k1 = 2 * pr, 2 * pr + 1
                                j0, j1 = k0 - NH * qb, k1 - NH * qb
                                q00 = P * j0 if j0 >= 0 else 0
                                q01 = P * j1 if j1 >= 0 else 0
                                ps = ps_s.tile([P, 2, QB], f32, tag="ps")
                                nc.tensor.matmul(
                                    ps[:, 0, q00:QB],
                                    KTB[k0 // 4][:, (k0 % 4) * P:
                                                 (k0 % 4 + 1) * P],
                                    QTB[qb][:, h, q00:QB],
                                    start=True, stop=True)
                                nc.tensor.matmul(
                                    ps[:, 1, q01:QB],
                                    KTB[k1 // 4][:, (k1 % 4) * P:
                                                 (k1 % 4 + 1) * P],
                                    QTB[qb][:, h, q01:QB],
                                    start=True, stop=True)
                                ex = expool.tile([P, 2, QB], bf16, tag="ex")
                                # one exp per full pair; diagonal halves get
                                # exact ranges (never read unwritten PSUM)
                                if j0 >= 0:
                                    nc.scalar.activation(
                                        ex[:, 0, q00:QB], ps[:, 0, q00:QB],
                                        AF.Exp)
                                    nc.scalar.activation(
                                        ex[:, 1, q01:QB], ps[:, 1, q01:QB],
                                        AF.Exp)
                                else:
                                    nc.scalar.activation(ex[:], ps[:], AF.Exp)
                                # zero dead regions + triangular mask on diag
                                for half, (jj, qq) in enumerate(
                                        ((j0, q00), (j1, q01))):
                                    if jj >= 0:
                                        if qq > 0:
                                            nc.gpsimd.memset(ex[:, half, 0:qq],
                                                             0.0)
                                        nc.vector.tensor_tensor(
                                            ex[:, half, qq:qq + P],
                                            ex[:, half, qq:qq + P],
                                            tri_t[:], MUL)
                                nc.tensor.matmul(
                                    oT[:, q00:QB], VS[:, k0], ex[:, 0, q00:QB],
                                    start=(k0 == 0), stop=False)
                                nc.tensor.matmul(
                                    oT[:, q01:QB], VS[:, k1], ex[:, 1, q01:QB],
                                    start=False, stop=(k1 == nk - 1))
                                ex_tiles.append(ex)

                                # each 2 pairs -> denominator group matmul
                                if pr % 2 == 1:
                                    g = pr // 2
                                    exa, exb = ex_tiles
                                    ex_tiles = []
                                    s01 = sumpool.tile([P, QB], bf16, tag="s01")
                                    nc.vector.tensor_tensor(
                                        s01[:], exa[:, 0], exa[:, 1], ADD)
                                    s23 = sumpool.tile([P, QB], bf16, tag="s23")
                                    nc.vector.tensor_tensor(
                                        s23[:], exb[:, 0], exb[:, 1], ADD)
                                    s03 = sumpool.tile([P, QB], bf16, tag="s03")
                                    nc.vector.tensor_tensor(
                                        s03[:], s01[:], s23[:], ADD)
                                    nc.tensor.matmul(
                                        den[:], ones_t[:], s03[:],
                                        start=(g == 0), stop=(g == ngrp - 1))

                            rec = recpool.tile([P, QB], f32, tag="rec")
                            nc.vector.reciprocal_approx_fast(rec[:], den[:])
                            nc.vector.tensor_tensor(
                                YTB[qb][:, h, :], oT[:], rec[:], MUL)

                            # phase-3 chunk for the PREVIOUS s-block fills
                            # the PE gap while DVE runs rec / normalize
                            if qb > 0:
                                for et in range(4 * h, 4 * h + 4):
                                    p3_unit(qb - 1, et)

                    for et in range(DT):
                        p3_unit(NQB - 1, et)

    nc.compile()
    return nc


def _fp8_split(a):
    """hi/lo fp8e4m3 split at a common scale: a ~= hi + lo (both fp8)."""
    import ml_dtypes
    f8 = ml_dtypes.float8_e4m3
    hi = a.astype(f8)
    lo = (a - hi.astype(np.float32)).astype(f8)
    return hi, lo


def _host_inputs(x, Wq, Wk, Wv, Wproj, q_gain):
    """Build the 8 per-core input maps (fp8 hi/lo x & qkv weights, bf16 rest)."""
    import ml_dtypes
    bf16 = ml_dtypes.bfloat16
    f32 = np.float32
    inv_freq = 1.0 / (ROPE_BASE ** (np.arange(0, HD, 2, dtype=f32) / HD))
    freqs = np.outer(np.arange(S, dtype=f32), inv_freq).astype(f32)
    c = np.cos(freqs).astype(f32)
    s = np.sin(freqs).astype(f32)
    cos2 = np.concatenate([c, c], axis=1).astype(bf16)
    sin2 = np.concatenate([s, -s], axis=1).astype(bf16)
    tri = np.triu(np.ones((P, P), dtype=f32)).astype(bf16)  # tri[k, q] = k <= q
    onesd = np.ones((P, P), dtype=bf16)
    ident = np.eye(P, dtype=f32).astype(bf16)

    in_maps = []
    for core in range(8):
        b, g = divmod(core, KVH)
        hs = g * NH * HD            # first q row for this group
        qg = np.full((NH + 1,), 1.0 / 1024, dtype=f32)
        qg[0:NH] = (q_gain[g * NH:(g + 1) * NH].astype(f32)
                    * (HD ** -0.5) / 1024)
        xTh, xTl = _fp8_split(
            np.ascontiguousarray(x[b].T, dtype=f32) * FSC)
        wqTh, wqTl = _fp8_split(
            np.ascontiguousarray(Wq[hs:hs + NH * HD].T, dtype=f32) * FSC)
        wkvTh, wkvTl = _fp8_split(
            np.ascontiguousarray(np.concatenate(
                [Wk[g * HD:(g + 1) * HD], Wv[g * HD:(g + 1) * HD]],
                axis=0).T, dtype=f32) * FSC)
        in_maps.append({
            "xTh": xTh, "xTl": xTl,
            "wqTh": wqTh, "wqTl": wqTl,
            "wkvTh": wkvTh, "wkvTl": wkvTl,
            "wpT": np.ascontiguousarray(Wproj.T[hs:hs + NH * HD]).astype(bf16),
            "cos2": cos2, "sin2": sin2,
            "qg5": np.ascontiguousarray(np.broadcast_to(qg, (P, NH + 1)),
                                        dtype=f32),
            "tri": tri, "onesd": onesd, "ident": ident,
        })
    return in_maps


def kernel(x, Wq, Wk, Wv, Wproj, q_gain):
    from concourse.bass_utils import run_bass_kernel_spmd

    x = np.asarray(x, dtype=np.float32)
    Wq = np.asarray(Wq, dtype=np.float32)
    Wk = np.asarray(Wk, dtype=np.float32)
    Wv = np.asarray(Wv, dtype=np.float32)
    Wproj = np.asarray(Wproj, dtype=np.float32)
    q_gain = np.asarray(q_gain, dtype=np.float32)

    if "nc" not in _CACHE:
        _CACHE["nc"] = _build_nc()
    nc = _CACHE["nc"]

    in_maps = _host_inputs(x, Wq, Wk, Wv, Wproj, q_gain)
    res = run_bass_kernel_spmd(nc, in_maps, core_ids=list(range(8)))

    out = np.zeros((B, S, D), dtype=np.float32)
    for core in range(8):
        b = core // KVH
        out[b] += res.results[core]["outT"].T.astype(np.float32)
    return out
